# revision 2
# baseline (speedup 1.0000x reference)
"""Trainium2 Bass kernel for nn_AlphaEntmax (sparsemax via clamped alpha=2).

The module's ClampMin/ClampMax composition maps any alpha in [1,2] to
exactly 2.0, so the reference computes sparsemax per row:
    p = relu(x - tau),  tau s.t. sum(relu(x - tau)) = 1.

Algorithm per row (K=1024): Newton/Michelot iterations for tau with
schedule R T R R A from tau0 = bf16_rowmax - 1.01 (a guaranteed lower
bound; from-left Newton never overshoots):
  - R slots: bf16 count-pass (is_gt, 4x DVE mode with add-accum) + bf16
    r-pass via the identity
        sum relu(x - tau) = sum(x) - sum min(x, tau)
    Both accumulators stay |.|<~40, so the sequential f32 accumulation is
    exact to ~1e-4. (Accumulating sum max(x,tau) ~ 2500 instead loses
    ~5e-2 to rounding, and the custom relu-reduce DVE op runs 1x = 3.5x
    slower; the accumulator reduces the op0 intermediate, so a fused
    relu+sum tensor_scalar does not exist.)
  - T slot: count only; r updated by the trapezoid estimate
        r -= step_prev*(c + c_prev)/2
  - A slot (anchor): exact f32 r on ScalarE (activation Relu + accum) and
    an is_ge bf16 count. is_ge overcounts at bf16 ties, making the final
    Newton step undershoot-safe; with the exact f32 r the anchor lands on
    tau* exactly whenever the final linear segment was reached.
    Measured absmax vs the reference 50-iter f32 bisection: 7.8e-3
    (gate 2e-2).
  - final: p = relu(x - tau) on ScalarE, written in bf16 (host upcasts;
    output values are in [0,1] so bf16 adds at most ~2e-3).

Engine split per tile [128,1024] (per-pass engine-busy ns):
  ACT: f32->bf16 cast via Copy (shares the Relu table, ~1040), anchor
       r-pass (~1225), final relu (~1040)
  DVE: bf16 rowmax + rowsum (2x327), 4 bf16 counts + 2 bf16 min-passes +
       1 is_ge (7x327), [128,8] update chains batched per wave
  DMA: f32 in (512KB/tile), bf16 out (256KB/tile); floor ~145us
Waves of 2 groups x G=4 tiles emit with passes interleaved at tile
granularity (hides accum WAW sem latency); phase-A (DVE iterations) of
wave w+1 is emitted before phase-B (ACT anchor/final) of wave w so the
engines overlap across waves. TimelineSim: ~272us vs 294us baseline.

Sharding: x [8,16,512,1024] split along batch, one entry per core.
"""

import numpy as np

B, H, Q, K = 8, 16, 512, 1024
N_CORES = 8
P = 128
ROWS_PER_CORE = (B // N_CORES) * H * Q  # 8192
N_TILES = ROWS_PER_CORE // P  # 64
G = 4  # tiles per group (batched update chains)
INTERLEAVE = 2  # groups emitted in lockstep
SCHED = ["R", "T", "R", "R", "A"]
TAU0_GUARD = 1.01  # covers bf16 rounding of the row max
FINAL_SPLIT = 1024  # columns of the final pass on ACT (rest on DVE)

_NC_CACHE = None


def _build_nc():
    import concourse.bacc as bacc
    import concourse.mybir as mybir
    from concourse.tile import TileContext

    f32 = mybir.dt.float32
    bf16 = mybir.dt.bfloat16
    Alu = mybir.AluOpType
    Act = mybir.ActivationFunctionType

    nc = bacc.Bacc(
        "TRN2", target_bir_lowering=False, debug=False, num_devices=N_CORES
    )
    x_ext = nc.dram_tensor("x", [ROWS_PER_CORE, K], f32, kind="ExternalInput")
    out_ext = nc.dram_tensor("out", [ROWS_PER_CORE, K], bf16, kind="ExternalOutput")

    N_GROUPS = N_TILES // G
    GK = G * K
    n_iter = len(SCHED)
    with TileContext(nc) as tc:
        with (
            tc.tile_pool(name="xp", bufs=6) as xp,
            tc.tile_pool(name="bp", bufs=6) as bp,
            tc.tile_pool(name="op", bufs=3) as op,
            tc.tile_pool(name="scr", bufs=1) as scr,
            tc.tile_pool(name="st", bufs=12) as st,
        ):
            # ping-pong elementwise scratches (outputs nobody reads)
            scrV = [scr.tile([P, K], bf16, tag=f"scrV{i}", name=f"scrV{i}")
                    for i in range(4)]
            scrS = [scr.tile([P, K], f32, tag=f"scrS{i}", name=f"scrS{i}")
                    for i in range(2)]
            vq = [0]

            def vscr():
                t = scrV[vq[0] % 4]
                vq[0] += 1
                return t

            # warm the ACT table during the first DMA
            nc.scalar.activation(
                scrS[0][:, :1], nc.const_aps.aps[(f32, 0.0)], Act.Relu
            )

            W = INTERLEAVE * G  # stats batched across the whole wave

            def emit_load(g):
                rows = slice(g * G * P, (g + 1) * G * P)
                x_dram = x_ext.ap()[rows, :].rearrange("(t p) k -> p t k", p=P)
                xb = xp.tile([P, GK], f32, tag="xb")
                xbf = bp.tile([P, GK], bf16, tag="xbf")
                for i in range(G):
                    nc.sync.dma_start(
                        out=xb[:, i * K : (i + 1) * K], in_=x_dram[:, i, :]
                    )
                return xb, xbf

            def alloc_stats():
                return {
                    n: st.tile([P, W], f32, tag=n, name=n)
                    for n in ("mx", "tau", "ntau", "r", "c0", "c1",
                              "rc", "stp", "cs", "sm", "sx")
                }

            def emit_cast(i, xb, xbf, stt, j0):
                # cast f32->bf16 on ScalarE (Copy shares the Relu table);
                # row max + row sum from cheap bf16 4x passes on DVE
                nc.scalar.activation(
                    xbf[:, i * K : (i + 1) * K], xb[:, i * K : (i + 1) * K],
                    Act.Copy,
                )
                nc.vector.tensor_scalar(
                    vscr()[:], xbf[:, i * K : (i + 1) * K], 0.0, None,
                    Alu.add, Alu.max,
                    accum_out=stt["mx"][:, j0 + i : j0 + i + 1],
                )
                # sx = sum of the bf16 shadow (for r = sx - sum min(x,tau))
                nc.vector.tensor_scalar(
                    vscr()[:], xbf[:, i * K : (i + 1) * K], 0.0, None,
                    Alu.add, Alu.add,
                    accum_out=stt["sx"][:, j0 + i : j0 + i + 1],
                )

            def emit_tau0(stt):
                nc.vector.tensor_scalar(
                    stt["tau"][:], stt["mx"][:], -TAU0_GUARD, None, Alu.add
                )
                nc.vector.tensor_scalar(
                    stt["ntau"][:], stt["tau"][:], -1.0, None, Alu.mult
                )

            def emit_pass(it, i, xb, xbf, stt, j0):
                kind = SCHED[it]
                c = stt["c0"] if it % 2 == 0 else stt["c1"]
                xcol = xb[:, i * K : (i + 1) * K]
                xfcol = xbf[:, i * K : (i + 1) * K]
                ji = j0 + i
                tau_i = stt["tau"][:, ji : ji + 1]
                if kind == "A":
                    # overshoot-safe bf16 count (is_ge overcounts ties)
                    nc.vector.tensor_scalar(
                        vscr()[:], xfcol, tau_i, None, Alu.is_ge, Alu.add,
                        accum_out=c[:, ji : ji + 1],
                    )
                    # exact f32 r on ScalarE
                    nc.scalar.activation(
                        scrS[i % 2][:], xcol, Act.Relu,
                        bias=stt["ntau"][:, ji : ji + 1],
                        accum_out=stt["r"][:, ji : ji + 1],
                    )
                else:
                    nc.vector.tensor_scalar(
                        vscr()[:], xfcol, tau_i, None, Alu.is_gt, Alu.add,
                        accum_out=c[:, ji : ji + 1],
                    )
                    if kind == "R":
                        # sm = sum min(x_bf, tau); r = sx - sm. Small-value
                        # accumulators (|.|~35) keep the f32 sums exact to
                        # ~1e-4, unlike sum max(x,tau) (~2500, ~5e-2 noise)
                        nc.vector.tensor_scalar(
                            vscr()[:], xfcol, tau_i, None, Alu.min, Alu.add,
                            accum_out=stt["sm"][:, ji : ji + 1],
                        )

            def emit_update(it, stt):
                kind = SCHED[it]
                c = stt["c0"] if it % 2 == 0 else stt["c1"]
                c_prev = stt["c1"] if it % 2 == 0 else stt["c0"]
                if kind == "R":
                    nc.vector.tensor_tensor(
                        stt["r"][:], stt["sx"][:], stt["sm"][:], Alu.subtract
                    )
                if kind == "T":
                    # r -= stp*(c + c_prev)/2
                    nc.vector.tensor_tensor(
                        stt["cs"][:], c[:], c_prev[:], Alu.add
                    )
                    nc.vector.tensor_tensor(
                        stt["cs"][:], stt["cs"][:], stt["stp"][:], Alu.mult
                    )
                    nc.vector.scalar_tensor_tensor(
                        stt["r"][:], stt["cs"][:], -0.5, stt["r"][:],
                        Alu.mult, Alu.add,
                    )
                # stp = (r-1)/c ; tau += stp
                nc.vector.reciprocal(stt["rc"][:], c[:])
                nc.vector.scalar_tensor_tensor(
                    stt["stp"][:], stt["r"][:], -1.0, stt["rc"][:],
                    Alu.add, Alu.mult,
                )
                nc.vector.tensor_tensor(
                    stt["tau"][:], stt["tau"][:], stt["stp"][:], Alu.add
                )
                nc.vector.tensor_scalar(
                    stt["ntau"][:], stt["tau"][:], -1.0, None, Alu.mult
                )

            def emit_final(g, xb, stt, j0):
                rows = slice(g * G * P, (g + 1) * G * P)
                o_dram = out_ext.ap()[rows, :].rearrange("(t p) k -> p t k", p=P)
                ob = op.tile([P, GK], bf16, tag="ob")
                s = FINAL_SPLIT
                for i in range(G):
                    xcol = xb[:, i * K : (i + 1) * K]
                    ocol = ob[:, i * K : (i + 1) * K]
                    ji = j0 + i
                    ntau_i = stt["ntau"][:, ji : ji + 1]
                    if s > 0:
                        nc.scalar.activation(
                            ocol[:, :s], xcol[:, :s], Act.Relu, bias=ntau_i
                        )
                    if s < K:
                        nc.vector.tensor_scalar(
                            ocol[:, s:], xcol[:, s:], stt["tau"][:, ji : ji + 1],
                            0.0, Alu.subtract, Alu.max,
                        )
                    nc.sync.dma_start(
                        out=o_dram[:, i, :], in_=ocol
                    )

            def emit_phase_a(w):
                """DVE-heavy prefix: load, cast+max, iterations 0..n-2."""
                gs = [w * INTERLEAVE + j for j in range(INTERLEAVE)]
                state = [emit_load(g) for g in gs]
                stt = alloc_stats()
                for i in range(G):
                    for jg, (xb, xbf) in enumerate(state):
                        emit_cast(i, xb, xbf, stt, jg * G)
                emit_tau0(stt)
                for it in range(n_iter - 1):
                    for i in range(G):
                        for jg, (xb, xbf) in enumerate(state):
                            emit_pass(it, i, xb, xbf, stt, jg * G)
                    emit_update(it, stt)
                return gs, state, stt

            def emit_phase_b(gs, state, stt):
                """ACT-heavy tail: anchor passes, final update, final, store."""
                it = n_iter - 1
                for i in range(G):
                    for jg, (xb, xbf) in enumerate(state):
                        emit_pass(it, i, xb, xbf, stt, jg * G)
                emit_update(it, stt)
                for jg, (g, (xb, xbf)) in enumerate(zip(gs, state)):
                    emit_final(g, xb, stt, jg * G)

            # software pipeline: phase-A of wave w+1 is emitted before
            # phase-B of wave w so ACT anchors/finals of w overlap the DVE
            # iteration block of w+1
            assert N_GROUPS % INTERLEAVE == 0
            n_waves = N_GROUPS // INTERLEAVE
            prev = emit_phase_a(0)
            for w in range(1, n_waves):
                cur = emit_phase_a(w)
                emit_phase_b(*prev)
                prev = cur
            emit_phase_b(*prev)

    nc.compile()
    return nc


def _get_nc():
    global _NC_CACHE
    if _NC_CACHE is None:
        _NC_CACHE = _build_nc()
    return _NC_CACHE


def _effective_alpha(alpha):
    a = np.asarray(alpha, dtype=np.float32)
    a = np.maximum(np.minimum(a, 0.0) - 1.0, 0.0) + 1.0 + np.maximum(a, 0.0)
    a = np.minimum(np.maximum(a, 0.0) - 2.0, 0.0) + 2.0 + np.minimum(a, 0.0)
    return a.astype(np.float32)


def _entmax_bisect_numpy(x, a, n_iter=50):
    """Generic-alpha fallback replicating the reference bisection in f32.
    Never taken for alpha in [1,2] (the clamp maps those to exactly 2.0)."""
    f32 = np.float32
    X = x.reshape(-1, K).astype(np.float32)
    am1 = (np.broadcast_to(a.reshape(1, H), (B, H)).reshape(-1)[
        np.arange(X.shape[0]) // Q
    ].astype(np.float32) - f32(1.0))[:, None]
    Xs = (X * am1).astype(np.float32)

    def p(s):
        pos = s > 0
        return np.where(
            pos, np.power(np.where(pos, s, f32(1.0)), (f32(1.0) / am1)), f32(0.0)
        ).astype(np.float32)

    mx = Xs.max(axis=1, keepdims=True).astype(np.float32)
    tau_lo = (mx - f32(1.0)).astype(np.float32)
    tau_hi = (mx - np.power(f32(1.0 / K), am1)).astype(np.float32)
    f_lo = (p(Xs - tau_lo).sum(axis=1, dtype=np.float32)[:, None] - f32(1.0)).astype(
        np.float32
    )
    dm = (tau_hi - tau_lo).astype(np.float32)
    tau_m = tau_lo.copy()
    for _ in range(n_iter):
        dm = (dm * f32(0.5)).astype(np.float32)
        tau_m = (tau_lo + dm).astype(np.float32)
        f_m = (p(Xs - tau_m).sum(axis=1, dtype=np.float32)[:, None] - f32(1.0)).astype(
            np.float32
        )
        tau_lo = np.where(f_m * f_lo >= 0, tau_m, tau_lo).astype(np.float32)
    pm = p(Xs - tau_m)
    s = pm.sum(axis=1, dtype=np.float32).astype(np.float32)[:, None]
    return (pm / s).astype(np.float32).reshape(B, H, Q, K)


def kernel(**inputs) -> np.ndarray:
    from concourse.bass_utils import run_bass_kernel_spmd

    x = np.ascontiguousarray(np.asarray(inputs["x"], dtype=np.float32))
    alpha = np.asarray(inputs.get("alpha", np.full((1, H), 1.5, np.float32)))
    a_eff = _effective_alpha(alpha)
    if not np.all(a_eff == np.float32(2.0)):
        return _entmax_bisect_numpy(x, a_eff)

    shards = x.reshape(N_CORES, ROWS_PER_CORE, K)
    in_maps = [{"x": shards[i]} for i in range(N_CORES)]

    nc = _get_nc()
    res = run_bass_kernel_spmd(nc, in_maps, core_ids=list(range(N_CORES)))
    out = np.stack(
        [np.asarray(res.results[i]["out"], dtype=np.float32) for i in range(N_CORES)]
    )
    return out.reshape(B, H, Q, K)
